# revision 22
# baseline (speedup 1.0000x reference)
"""Depth-aware 3x3 conv (depth-similarity modulated conv) on 8 Trainium2
NeuronCores, batch-parallel (1 image per core).

out[b,o,h,w] = sum_{c,k} weight[o,c,k] * fd[b,k,h,w] * xpatch[b,c,k,h,w] + bias
fd[k,p] = exp(-8.3 * |depth[p + delta_k] - depth[p]|)   (zero-padded patches)

v3 design (per core, image [64, 256, 256] fp16):
- Padded plane flattened: q = (h+1)*258 + (w+1), NP = 258*258.
- 8 modulated taps in 4 partition-pair tiles (2 taps x 64ch = K=128 matmuls),
  center tap unmodulated K=64. Pair x tiles (xs := x line at XSL+q0-260):
    T1 = [xs@+1 ; xs@+2]   (2 HBM streams)     A=(t0,t1) mb=0 s=0, B=(t7,t8)
                                               mb=516 s=1
    T2 = [xs@+1 ; xs@+257] (DVE 4x copies)     C=(t2,t3) mb=2,  D=(t5,t6)
                                               mb=260  (s=0)
  (mb = even modulate base so DVE tensor_tensor runs in 2x mode; s = extra
  column offset applied at the matmul read.)
- fd packed [96, 512] (= 8 taps x 12 segs; A/B taps tap-major for DRAM line
  writes, C/D taps seg-major), sub on DVE, abs+exp on ACT.
- fd fanout to 64 channel rows:
    pairs A,B: DRAM round-trip (4 lines) + one [128, CH] stride-0 DMA per pair
    pairs C,D: per-1024-wave PE broadcast matmul (lhsT = E2 ones-blocks, rhs =
      fdcd rows) -> PSUM [128,1024], ACT copy -> SBUF fp16. No DMA.
- Modulate: A + C/D on DVE (2x), B on GPSIMD.
- Mains per 512-group: 5 accumulating matmuls; bcasts emitted one wave ahead;
  ScalarE evicts with bias -> fp16.
"""
import numpy as np

import concourse.bacc as bacc
import concourse.bass as bass
import concourse.mybir as mybir
import concourse.tile as tile
from concourse.bass_utils import run_bass_kernel_spmd

F16 = mybir.dt.float16
F32 = mybir.dt.float32

B, C, H, W = 8, 64, 256, 256
Hp, Wp = H + 2, W + 2          # 258
NP = Hp * Wp                   # 66564
ALPHA = 8.3

GW = 512                       # matmul group width (psum bank)
SEGS = 12                      # fd segments / groups per chunk
CH = GW * SEGS                 # 6144 output pixels per chunk
NCHUNK = 11                    # 11*6144 = 67584 >= NP
OUTW = NCHUNK * CH
HCH = CH // 2                  # 3072 (half-chunk)

XSL, XSH = 512, 4608           # x line slacks (elements)
DSL, DSH = 512, 4608
XW = XSL + NP + XSH
DW = DSL + NP + DSH

T1_W = CH + 518                # 6662
T2_W = CH + 262                # 6406

FD_SL = 512
LW = FD_SL + OUTW + 512        # fd DRAM line width


def _build_nc():
    nc = bacc.Bacc("TRN2", target_bir_lowering=False, debug=False, num_devices=8)
    x_line = nc.declare_dram_parameter("x_line", [C, XW], F16, isOutput=False)
    d_line = nc.declare_dram_parameter("d_line", [1, DW], F32, isOutput=False)
    wts = nc.declare_dram_parameter("wts", [128, 448], F16, isOutput=False)
    bias = nc.declare_dram_parameter("bias", [64, 1], F32, isOutput=False)
    out_l = nc.declare_dram_parameter("out_line", [C, OUTW], F16, isOutput=True)

    x_t = x_line.ap().tensor
    d_t = d_line.ap().tensor
    fd_dram = nc.dram_tensor("fd_scratch", [4, LW], F16)
    fd_t = fd_dram.ap().tensor

    with tile.TileContext(nc) as tc:
        with (
            tc.tile_pool(name="const", bufs=1) as cpool,
            tc.tile_pool(name="xt", bufs=2) as xpool,
            tc.tile_pool(name="fdgen", bufs=2) as gpool,
            tc.tile_pool(name="fr", bufs=2) as fpool,
            tc.tile_pool(name="frsb", bufs=1) as fspool,
            tc.tile_pool(name="mmod", bufs=2) as mpool,
            tc.tile_pool(name="mcd", bufs=2) as mcdpool,
            tc.tile_pool(name="ost", bufs=2) as opool,
            tc.tile_pool(name="ps", bufs=2, space="PSUM") as pspool,
            tc.tile_pool(name="psfr", bufs=2, space="PSUM") as pfpool,
        ):
            wt_sb = cpool.tile([128, 448], F16, tag="w")
            nc.sync.dma_start(wt_sb[:], wts[:])
            bias_sb = cpool.tile([64, 1], F32, tag="b")
            nc.sync.dma_start(bias_sb[:], bias[:])

            for i in range(NCHUNK):
                q0 = i * CH
                xbase = XSL + q0 - 260

                # ---- x tiles: T1 = [xs@+1; xs@+2] (HBM), T2 via DVE copies --
                t1 = xpool.tile([128, T1_W], F16, tag="t1")
                nc.sync.dma_start(
                    t1[0:64, :],
                    bass.AP(x_t, xbase + 1, [[XW, 64], [1, T1_W]]))
                nc.sync.dma_start(
                    t1[64:128, :],
                    bass.AP(x_t, xbase + 2, [[XW, 64], [1, T1_W]]))
                t2 = xpool.tile([128, T2_W], F16, tag="t2")
                nc.vector.tensor_copy(t2[0:64, :], t1[0:64, 0:T2_W])
                nc.vector.tensor_copy(t2[64:128, :],
                                      t1[0:64, 256:256 + T2_W])

                # ---- fd generation, packed [96, 512]:
                #   A/B taps tap-major: t0 0:12, t1 12:24, t7 24:36, t8 36:48
                #   C/D taps seg-major: 48+4g+{0:t2, 1:t3, 2:t5, 3:t6}
                dp = gpool.tile([96, GW], F32, tag="dp")
                nc.sync.dma_start(
                    dp[0:24, :],
                    bass.AP(d_t, DSL + q0 - 259,
                            [[1, 2], [GW, SEGS], [1, GW]]))
                nc.sync.dma_start(
                    dp[24:48, :],
                    bass.AP(d_t, DSL + q0 + 258,
                            [[1, 2], [GW, SEGS], [1, GW]]))
                for j, dlt in enumerate((-257, -1, 1, 257)):
                    nc.sync.dma_start(
                        dp[48 + j:96:4, :],
                        bass.AP(d_t, DSL + q0 + dlt, [[GW, SEGS], [1, GW]]))
                dc = gpool.tile([96, GW], F32, tag="dc")
                nc.sync.dma_start(
                    dc[0:48, :],
                    bass.AP(d_t, DSL + q0,
                            [[0, 4], [GW, SEGS], [1, GW]]))
                nc.sync.dma_start(
                    dc[48:96, :],
                    bass.AP(d_t, DSL + q0,
                            [[GW, SEGS], [0, 4], [1, GW]]))
                df = gpool.tile([96, GW], F32, tag="df", bufs=1)
                nc.vector.tensor_tensor(df[:], dp[:], dc[:],
                                        mybir.AluOpType.subtract)
                da = gpool.tile([96, GW], F32, tag="da", bufs=1)
                nc.scalar.activation(da[:], df[:],
                                     mybir.ActivationFunctionType.Abs)
                fdp = gpool.tile([96, GW], F16, tag="fdp")
                nc.scalar.activation(fdp[:], da[:],
                                     mybir.ActivationFunctionType.Exp,
                                     scale=-ALPHA)

                # ---- pairs A,B: fd lines to DRAM, stride-0 fanout DMA ----
                for l in range(4):
                    nc.sync.dma_start(
                        bass.AP(fd_t, l * LW + FD_SL + q0,
                                [[GW, SEGS], [1, GW]]),
                        fdp[l * SEGS:(l + 1) * SEGS, :])
                frA = fpool.tile([128, CH], F16, tag="frA")
                nc.sync.dma_start(
                    frA[:],
                    bass.AP(fd_t, FD_SL + q0,
                            [[LW, 2], [0, 64], [1, CH]]))
                frB = fpool.tile([128, CH + 2], F16, tag="frB")
                nc.sync.dma_start(
                    frB[:],
                    bass.AP(fd_t, 2 * LW + FD_SL + q0 - 1,
                            [[LW, 2], [0, 64], [1, CH + 2]]))

                # ---- pairs C,D: reshape fd rows into [34, CH] (D at 32) ----
                fdcd = gpool.tile([34, CH], F16, tag="fdcd", bufs=1)
                for j, dst in enumerate((fdcd[0:1, :], fdcd[1:2, :],
                                         fdcd[32:33, :], fdcd[33:34, :])):
                    nc.sync.dma_start(dst, fdp[48 + j:96:4, :])

                # ---- modulate A (DVE) and B (GPSIMD), half-chunk ops ----
                mtA, mtB = [], []
                for h in range(2):
                    ma = mpool.tile([128, HCH], F16, tag=f"mtA{h}")
                    nc.vector.tensor_tensor(
                        ma[:], t1[:, h * HCH:(h + 1) * HCH],
                        frA[:, h * HCH:(h + 1) * HCH], mybir.AluOpType.mult)
                    mtA.append(ma)
                    mb = mpool.tile([128, HCH + 2], F16, tag=f"mtB{h}")
                    nc.gpsimd.tensor_tensor(
                        mb[:], t1[:, 516 + h * HCH:516 + h * HCH + HCH + 2],
                        frB[:, h * HCH:h * HCH + HCH + 2],
                        mybir.AluOpType.mult)
                    mtB.append(mb)

                # ---- waves: bcast C/D + copy + modulate; mains trail 1 wave -
                osts = [opool.tile([64, HCH], F16, tag="o",
                                   name=f"ost{h}")
                        for h in range(2)]
                mcds = {}

                def mains(g):
                    h, lo = (0, 0) if g < 6 else (1, HCH)
                    ps = pspool.tile([64, GW], F32)
                    nc.tensor.matmul(
                        ps[:], wt_sb[:, 0:64],
                        mtA[h][:, g * GW - lo:(g + 1) * GW - lo],
                        start=True, stop=False)
                    nc.tensor.matmul(
                        ps[:], wt_sb[:, 64:128],
                        mtB[h][:, 1 + g * GW - lo:1 + (g + 1) * GW - lo],
                        start=False, stop=False)
                    mc, md = mcds.pop(g)
                    nc.tensor.matmul(
                        ps[:], wt_sb[:, 128:192], mc[:],
                        start=False, stop=False)
                    nc.tensor.matmul(
                        ps[:], wt_sb[:, 192:256], md[:],
                        start=False, stop=False)
                    nc.tensor.matmul(
                        ps[:], wt_sb[0:64, 256:320],
                        t1[0:64, 259 + g * GW: 259 + (g + 1) * GW],
                        start=False, stop=True)
                    nc.scalar.activation(
                        osts[h][:, g * GW - lo:(g + 1) * GW - lo], ps[:],
                        mybir.ActivationFunctionType.Identity,
                        bias=bias_sb[:], scale=1.0)

                for w in range(SEGS // 2):
                    for pj, (pbase, mb0) in enumerate(((0, 2), (32, 260))):
                        fps = pfpool.tile([128, 2 * GW], F32)
                        for gg in range(2):
                            nc.tensor.matmul(
                                fps[:, gg * GW:(gg + 1) * GW],
                                wt_sb[pbase:pbase + 2, 320:448],
                                fdcd[pbase:pbase + 2,
                                     (2 * w + gg) * GW:(2 * w + gg + 1) * GW],
                                start=True, stop=True,
                                tile_position=(pbase, 0))
                        fsb = fspool.tile([128, 2 * GW], F16, tag=f"f{pj}")
                        nc.scalar.activation(
                            fsb[:], fps[:],
                            mybir.ActivationFunctionType.Identity)
                        mm = mcdpool.tile([128, 2 * GW], F16, tag=f"m{pj}")
                        nc.vector.tensor_tensor(
                            mm[:],
                            t2[:, mb0 + w * 2 * GW: mb0 + (w + 1) * 2 * GW],
                            fsb[:], mybir.AluOpType.mult)
                        for gg in range(2):
                            mcds.setdefault(2 * w + gg, [None, None])[pj] = \
                                mm[:, gg * GW:(gg + 1) * GW]
                    if w > 0:
                        mains(2 * w - 2)
                        mains(2 * w - 1)
                        if w == 3:
                            nc.sync.dma_start(
                                out_l[:, q0:q0 + HCH], osts[0][:])
                for g in (10, 11):
                    mains(g)
                nc.sync.dma_start(
                    out_l[:, q0 + HCH:q0 + CH], osts[1][:])
    nc.compile()
    return nc


_NC_CACHE = None


def _get_nc():
    global _NC_CACHE
    if _NC_CACHE is None:
        _NC_CACHE = _build_nc()
    return _NC_CACHE


def _make_in_maps(inputs):
    x = np.asarray(inputs["x"], dtype=np.float32)
    depth = np.asarray(inputs["depth"], dtype=np.float32)
    weight = np.asarray(inputs["weight"], dtype=np.float32)
    bias_np = np.asarray(inputs["bias"], dtype=np.float32)

    xl = np.zeros((B, C, XW), np.float16)
    xpad = np.zeros((B, C, Hp, Wp), np.float32)
    xpad[:, :, 1:257, 1:257] = x
    xl[:, :, XSL:XSL + NP] = xpad.reshape(B, C, NP).astype(np.float16)

    dl = np.zeros((B, 1, DW), np.float32)
    dpad = np.zeros((B, Hp, Wp), np.float32)
    dpad[:, 1:257, 1:257] = depth[:, 0]
    dl[:, 0, DSL:DSL + NP] = dpad.reshape(B, NP)

    wts = np.zeros((128, 448), np.float16)
    # pairs: A=(t0,t1), B=(t7,t8), C=(t2,t3), D=(t5,t6); lhsT[c,o] = w[o,c,k]
    for g, (ta, tb) in enumerate(((0, 1), (7, 8), (2, 3), (5, 6))):
        wts[0:64, g * 64:(g + 1) * 64] = \
            weight[:, :, ta // 3, ta % 3].T.astype(np.float16)
        wts[64:128, g * 64:(g + 1) * 64] = \
            weight[:, :, tb // 3, tb % 3].T.astype(np.float16)
    wts[0:64, 256:320] = weight[:, :, 1, 1].T.astype(np.float16)
    # E2 broadcast matrices: psum rows 0:64 <- rhs row 0, rows 64:128 <- row 1
    # (pair C's copy at partitions 0:2, pair D's at 32:34)
    wts[0, 320:384] = 1.0
    wts[1, 384:448] = 1.0
    wts[32, 320:384] = 1.0
    wts[33, 384:448] = 1.0

    bias_col = bias_np.reshape(64, 1)
    return [
        {"x_line": xl[b], "d_line": dl[b], "wts": wts, "bias": bias_col}
        for b in range(B)
    ]


def kernel(x, depth, weight, bias):
    nc = _get_nc()
    in_maps = _make_in_maps(
        {"x": x, "depth": depth, "weight": weight, "bias": bias})
    res = run_bass_kernel_spmd(nc, in_maps, list(range(B)))

    out = np.empty((B, C, H, W), np.float32)
    for b in range(B):
        ol = res.results[b]["out_line"][:, :NP].astype(np.float32)
        out[b] = ol.reshape(C, Hp, Wp)[:, 1:257, 1:257]
    return out
